# revision 22
# baseline (speedup 1.0000x reference)
"""Trainium2 Bass kernel for nn_AttnMatching.

Reference computes:
    emb = emb_table[1:L+1]                      # [L, D]
    attn = einsum('ld,ntd->nlt', emb, self_attn)
    out  = einsum('nlt,t->nl', attn, value_w[0])

Reassociated (identical math):
    ctx[n, d] = sum_t value_w[t] * self_attn[n, t, d]    # [N, D]  (tiny:
              #  0.1% of total FLOPs -> folded on host during marshalling)
    out[n, l] = sum_d ctx[n, d] * emb[l, d]              # [N, L]

Memory-bound: dominant traffic is streaming the embedding table. All
device traffic is bf16 (rel_norm vs fp32 reference ~3e-3, gate is 2e-2).

Sharding: vocab axis L split across 8 cores (6250 cols each), no
communication. Host-side marshalling per core: the ctxT [D,16] block and
each DMA chunk of the emb shard are separate contiguous DRAM tensors
(sequential HBM reads, 4 KB packets); outputs come back quad-interleaved
and are de-interleaved + upcast on host.

Per-core program (raw bacc, hand-rolled sems):
  - chunk loads issued in the entry block, spread over the sync/scalar
    HWDGE rings and the gpsimd SWDGE ring in need-order.
  - PE: bf16 warmup matmuls on scratch bridge the HAM clock-gate window,
    then mains with 4-way COLUMN TILING: matmul s uses
    tile_position col-group (s%4), so its [16,<=512] output lands at
    PSUM partitions 32q..32q+15 and four consecutive matmuls pack one
    full [128,512] PSUM bank. lhsT = ctxT [128,16] reloaded per matmul
    (16-col LDW, ~free); rhs = emb cols [128,<=512]. 13 matmuls fill
    banks 0..3 with no bank reuse.
  - One full-width [128,512] PSUM->SBUF bf16 copy per bank (DVE even
    banks, ACT odd banks) replaces 13 sixteenth-width copies.
  - stores: full-width [128,512] bf16 DMAs (one per bank, 16 engines),
    garbage quad rows included and stripped on host. No completion wait:
    the epilogue's clear_and_free dma_reset drains the store queue.
  - Epilogue: Block-exit barrier + semaphore clear (re-execution safe).
"""

import os

import numpy as np
import ml_dtypes

L = 50000
D = 128
T = 100
N = 16
NCORES = 8
LSH = L // NCORES          # 6250 columns per core
CTX = 16                   # ctxT [D, N] prepended to chunk 0
MM = 512                   # matmul moving-operand / PSUM bank limit
NQ = 4                     # col-group quads per PSUM bank

# knobs (env-overridable for A/B profiling)
DMA_CHUNK = int(os.environ.get("K_DMA_CHUNK", "1024"))  # emb load cols/chunk
N_WARMUP = int(os.environ.get("K_N_WARMUP", "8"))       # PE HAM warmup matmuls
LOAD_RINGS = os.environ.get("K_LOAD_RINGS", "sag")      # per-chunk ring cycle
STORE_RINGS = os.environ.get("K_STORE_RINGS", "sasa")   # per-bank store ring
NUM_DEVICES = int(os.environ.get("K_NUM_DEVICES", str(NCORES)))

_cache = {}


def _chunks(total, step, start=0):
    return [(c0, min(c0 + step, total)) for c0 in range(start, total, step)]


def _plan():
    # chunk i covers emb cols [a, b); chunk 0 also carries ctxT
    ch = [(0, MM)] + _chunks(LSH, DMA_CHUNK, start=MM)
    # split the ragged tail into its own tiny chunk on another ring so
    # the final matmul's gate lands concurrently with (not after) the
    # second-to-last chunk
    tail = LSH % MM
    if tail and ch[-1][1] - ch[-1][0] > tail:
        a, b = ch[-1]
        ch[-1] = (a, b - tail)
        ch.append((b - tail, b))
    mm_cols = _chunks(LSH, MM)
    gates = []
    for _c0, c1 in mm_cols:
        gates.append(next(i for i, (_a, b) in enumerate(ch) if b >= c1))
    n_mm = len(mm_cols)
    # bank b holds matmuls 4b..4b+3 (quads 0..3)
    banks = [(b0, min(b0 + NQ, n_mm)) for b0 in range(0, n_mm, NQ)]
    return ch, mm_cols, gates, banks


def _bank_shape(banks, mm_cols, b):
    """(partitions, cols) of bank b's useful region"""
    m0, m1 = banks[b]
    nq = m1 - m0
    cols = max(c1 - c0 for c0, c1 in mm_cols[m0:m1])
    return (128 if nq == NQ else nq * 32 - 16, cols)


def _build():
    import concourse.bacc as bacc
    import concourse.mybir as mybir

    f32 = mybir.dt.float32
    bf16 = mybir.dt.bfloat16

    nc = bacc.Bacc(
        "TRN2",
        target_bir_lowering=False,
        debug=False,
        enable_asserts=True,
        num_devices=NUM_DEVICES,
    )

    ch, mm_cols, gates, banks = _plan()
    nch = len(ch)
    n_mm = len(mm_cols)
    nbk = len(banks)

    # one contiguous DRAM tensor per load chunk / per-bank store
    emb_t = []
    for i, (a, b) in enumerate(ch):
        cols = (CTX if i == 0 else 0) + (b - a)
        emb_t.append(
            nc.dram_tensor(f"emb{i}", [D, cols], bf16, kind="ExternalInput").ap()
        )
    out_t = []
    for b in range(nbk):
        parts, cols = _bank_shape(banks, mm_cols, b)
        out_t.append(
            nc.dram_tensor(
                f"out{b}", [parts, cols], bf16, kind="ExternalOutput"
            ).ap()
        )

    embx_sb = nc.alloc_sbuf_tensor("embx_sb", [D, CTX + LSH], bf16).ap()
    out_sbs = []
    for b in range(nbk):
        parts, cols = _bank_shape(banks, mm_cols, b)
        out_sbs.append(
            nc.alloc_sbuf_tensor(f"out_sb{b}", [parts, cols], bf16).ap()
        )
    wscr = nc.alloc_sbuf_tensor("wscr", [D, CTX + MM], bf16).ap()
    ps = [
        nc.alloc_psum_tensor(f"ps{b}", [128, MM], f32).ap() for b in range(nbk)
    ]
    ps_w = nc.alloc_psum_tensor("ps_w", [128, MM], f32).ap()

    lde = [nc.alloc_semaphore(f"lde{i}") for i in range(nch)]
    mm_sem = nc.alloc_semaphore("mm")
    cpv = nc.alloc_semaphore("cpv")
    cpa = nc.alloc_semaphore("cpa")
    st = nc.alloc_semaphore("st")
    all_sems = lde + [mm_sem, cpv, cpa, st]

    eng = {"s": nc.sync, "a": nc.scalar, "g": nc.gpsimd}

    # entry block: all chunk loads, need-order ring heads
    for i, (a, b) in enumerate(ch):
        ring = eng[LOAD_RINGS[i % len(LOAD_RINGS)]]
        s0 = 0 if i == 0 else CTX + a
        s1 = CTX + b
        ring.dma_start(embx_sb[:, s0:s1], emb_t[i][:, :]).then_inc(lde[i], 16)

    # bank b copy engine: DVE for even b, ACT for odd b
    cp_eng = ["v" if b % 2 == 0 else "a" for b in range(nbk)]
    store_rings = [STORE_RINGS[b % len(STORE_RINGS)] for b in range(nbk)]

    def _bank_copy(b):
        parts, cols = _bank_shape(banks, mm_cols, b)
        src = ps[b][:parts, :cols]
        dst = out_sbs[b][:, :]
        if cp_eng[b] == "v":
            nc.vector.tensor_copy(dst, src).then_inc(cpv, 1)
        else:
            nc.scalar.copy(dst, src).then_inc(cpa, 1)

    def _bank_store(issuer, b):
        # stores wait for the last load so their data transfer never
        # contends with the emb stream (the epilogue drain covers their
        # completion), and for bank b's copy
        issuer.wait_ge(lde[nch - 1], 16)
        n_v = sum(1 for x in cp_eng[: b + 1] if x == "v")
        n_a = b + 1 - n_v
        if cp_eng[b] == "v":
            issuer.wait_ge(cpv, n_v)
        else:
            issuer.wait_ge(cpa, n_a)
        eng[store_rings[b]].dma_start(out_t[b][:, :], out_sbs[b][:, :]).then_inc(
            st, 16
        )

    # flat program (no nc.Block): per-engine streams emitted directly into
    # the entry block -- saves the per-engine branch + the Block-exit
    # drain-barrier. Only per-engine emission order matters.
    for _wi in range(N_WARMUP):
        nc.tensor.matmul(
            ps_w[:CTX, :],
            lhsT=wscr[:, :CTX],
            rhs=wscr[:, CTX:],
            start=True,
            stop=True,
        )
    prev_gate = -1
    for s, (c0, c1) in enumerate(mm_cols):
        if gates[s] != prev_gate:
            nc.tensor.wait_ge(lde[gates[s]], 16)
            prev_gate = gates[s]
        b, q = s // NQ, s % NQ
        nc.tensor.matmul(
            ps[b][32 * q : 32 * q + N, : c1 - c0],
            lhsT=embx_sb[:, :CTX],
            rhs=embx_sb[:, CTX + c0 : CTX + c1],
            start=True,
            stop=True,
            tile_position=(0, 32 * q),
        ).then_inc(mm_sem, 1)

    for b in range(nbk):
        if cp_eng[b] == "v":
            nc.vector.wait_ge(mm_sem, min(banks[b][1], n_mm))
            _bank_copy(b)

    for b in range(nbk):
        if cp_eng[b] == "a":
            nc.scalar.wait_ge(mm_sem, min(banks[b][1], n_mm))
            _bank_copy(b)
        if store_rings[b] == "a":
            _bank_store(nc.scalar, b)

    for b in range(nbk):
        if store_rings[b] == "s":
            _bank_store(nc.sync, b)

    # epilogue: sem-only all-engine barrier, then the sem clear
    # (re-execution safety); the clear's dma_reset drains the store
    # queue so the final writes land.
    nc.all_engine_barrier(sem_only=True)
    nc.clear_and_free_semaphores(all_sems)

    nc.compile()
    return nc


def _get_nc():
    if "nc" not in _cache:
        _cache["nc"] = _build()
    return _cache["nc"]


def _make_in_maps(self_attn, emb_table, value_w):
    bf = ml_dtypes.bfloat16
    sa = np.asarray(self_attn, dtype=np.float32)
    w = np.asarray(value_w, dtype=np.float32)[0]
    ctxT = np.einsum("ntd,t->dn", sa, w).astype(bf)          # [D, N]
    embT = np.asarray(emb_table, dtype=np.float32)[1 : L + 1].T.astype(bf)
    ch, _mm, _g, _banks = _plan()
    maps = []
    for k in range(NCORES):
        shard = embT[:, k * LSH : (k + 1) * LSH]
        m = {}
        for i, (a, b) in enumerate(ch):
            if i == 0:
                blk = np.concatenate([ctxT, shard[:, a:b]], axis=1)
            else:
                blk = shard[:, a:b]
            m[f"emb{i}"] = np.ascontiguousarray(blk)
        maps.append(m)
    return maps


def _unshard(results):
    _ch, mm_cols, _g, banks = _plan()
    full = np.empty((N, L), dtype=np.float32)
    for k in range(NCORES):
        base = k * LSH
        for b, (m0, m1) in enumerate(banks):
            blk = np.asarray(results[k][f"out{b}"]).astype(np.float32)
            for q, s in enumerate(range(m0, m1)):
                c0, c1 = mm_cols[s]
                full[:, base + c0 : base + c1] = blk[
                    32 * q : 32 * q + N, : c1 - c0
                ]
    return full


def run(self_attn, emb_table, value_w, trace=False):
    from concourse.bass_utils import run_bass_kernel_spmd

    nc = _get_nc()
    in_maps = _make_in_maps(self_attn, emb_table, value_w)
    res = run_bass_kernel_spmd(nc, in_maps, list(range(NCORES)), trace=trace)
    return _unshard(res.results), res


def kernel(self_attn, mat2, traj, emb_table, value_w):
    full, _ = run(self_attn, emb_table, value_w, trace=False)
    return full


# revision 23
# speedup vs baseline: 1.2489x; 1.2489x over previous
"""Trainium2 Bass kernel for nn_AttnMatching.

Reference computes:
    emb = emb_table[1:L+1]                      # [L, D]
    attn = einsum('ld,ntd->nlt', emb, self_attn)
    out  = einsum('nlt,t->nl', attn, value_w[0])

Reassociated (identical math):
    ctx[n, d] = sum_t value_w[t] * self_attn[n, t, d]    # [N, D]  (tiny:
              #  0.1% of total FLOPs -> folded on host during marshalling)
    out[n, l] = sum_d ctx[n, d] * emb[l, d]              # [N, L]

Memory-bound: dominant traffic is streaming the embedding table. All
device traffic is bf16 (rel_norm vs fp32 reference ~3e-3, gate is 2e-2).

Sharding: vocab axis L split across 8 cores (6250 cols each), no
communication. Host-side marshalling per core: the ctxT [D,16] block and
each DMA chunk of the emb shard are separate contiguous DRAM tensors
(sequential HBM reads, 4 KB packets); outputs come back quad-interleaved
and are de-interleaved + upcast on host.

Per-core program (raw bacc, hand-rolled sems):
  - chunk loads issued in the entry block, spread over the sync/scalar
    HWDGE rings and the gpsimd SWDGE ring in need-order.
  - PE: bf16 warmup matmuls on scratch bridge the HAM clock-gate window,
    then mains with 4-way COLUMN TILING: matmul s uses
    tile_position col-group (s%4), so its [16,<=512] output lands at
    PSUM partitions 32q..32q+15 and four consecutive matmuls pack one
    full [128,512] PSUM bank. lhsT = ctxT [128,16] reloaded per matmul
    (16-col LDW, ~free); rhs = emb cols [128,<=512]. 13 matmuls fill
    banks 0..3 with no bank reuse.
  - One full-width [128,512] PSUM->SBUF bf16 copy per bank (DVE even
    banks, ACT odd banks) replaces 13 sixteenth-width copies.
  - stores: full-width [128,512] bf16 DMAs (one per bank, 16 engines),
    garbage quad rows included and stripped on host. No completion wait:
    the epilogue's clear_and_free dma_reset drains the store queue.
  - Epilogue: Block-exit barrier + semaphore clear (re-execution safe).
"""

import os

import numpy as np
import ml_dtypes

L = 50000
D = 128
T = 100
N = 16
NCORES = 8
LSH = L // NCORES          # 6250 columns per core
CTX = 16                   # ctxT [D, N] prepended to chunk 0
MM = 512                   # matmul moving-operand / PSUM bank limit
NQ = 4                     # col-group quads per PSUM bank

# knobs (env-overridable for A/B profiling)
DMA_CHUNK = int(os.environ.get("K_DMA_CHUNK", "1024"))  # emb load cols/chunk
N_WARMUP = int(os.environ.get("K_N_WARMUP", "8"))       # PE HAM warmup matmuls
LOAD_RINGS = os.environ.get("K_LOAD_RINGS", "sag")      # per-chunk ring cycle
STORE_RINGS = os.environ.get("K_STORE_RINGS", "sasa")   # per-bank store ring
NUM_DEVICES = int(os.environ.get("K_NUM_DEVICES", str(NCORES)))

_cache = {}


def _chunks(total, step, start=0):
    return [(c0, min(c0 + step, total)) for c0 in range(start, total, step)]


def _plan():
    # chunk i covers emb cols [a, b); chunk 0 also carries ctxT
    ch = [(0, MM)] + _chunks(LSH, DMA_CHUNK, start=MM)
    mm_cols = _chunks(LSH, MM)
    gates = []
    for _c0, c1 in mm_cols:
        gates.append(next(i for i, (_a, b) in enumerate(ch) if b >= c1))
    n_mm = len(mm_cols)
    # bank b holds matmuls 4b..4b+3 (quads 0..3)
    banks = [(b0, min(b0 + NQ, n_mm)) for b0 in range(0, n_mm, NQ)]
    return ch, mm_cols, gates, banks


def _bank_shape(banks, mm_cols, b):
    """(partitions, cols) of bank b's useful region"""
    m0, m1 = banks[b]
    nq = m1 - m0
    cols = max(c1 - c0 for c0, c1 in mm_cols[m0:m1])
    return (128 if nq == NQ else nq * 32 - 16, cols)


def _build():
    import concourse.bacc as bacc
    import concourse.mybir as mybir

    f32 = mybir.dt.float32
    bf16 = mybir.dt.bfloat16

    nc = bacc.Bacc(
        "TRN2",
        target_bir_lowering=False,
        debug=False,
        enable_asserts=True,
        num_devices=NUM_DEVICES,
    )

    ch, mm_cols, gates, banks = _plan()
    nch = len(ch)
    n_mm = len(mm_cols)
    nbk = len(banks)

    # one contiguous DRAM tensor per load chunk / per-bank store
    emb_t = []
    for i, (a, b) in enumerate(ch):
        cols = (CTX if i == 0 else 0) + (b - a)
        emb_t.append(
            nc.dram_tensor(f"emb{i}", [D, cols], bf16, kind="ExternalInput").ap()
        )
    out_t = []
    for b in range(nbk):
        parts, cols = _bank_shape(banks, mm_cols, b)
        out_t.append(
            nc.dram_tensor(
                f"out{b}", [parts, cols], bf16, kind="ExternalOutput"
            ).ap()
        )

    embx_sb = nc.alloc_sbuf_tensor("embx_sb", [D, CTX + LSH], bf16).ap()
    out_sbs = []
    for b in range(nbk):
        parts, cols = _bank_shape(banks, mm_cols, b)
        out_sbs.append(
            nc.alloc_sbuf_tensor(f"out_sb{b}", [parts, cols], bf16).ap()
        )
    wscr = nc.alloc_sbuf_tensor("wscr", [D, CTX + MM], bf16).ap()
    ps = [
        nc.alloc_psum_tensor(f"ps{b}", [128, MM], f32).ap() for b in range(nbk)
    ]
    ps_w = nc.alloc_psum_tensor("ps_w", [128, MM], f32).ap()

    lde = [nc.alloc_semaphore(f"lde{i}") for i in range(nch)]
    mm_sem = nc.alloc_semaphore("mm")
    cpv = nc.alloc_semaphore("cpv")
    cpa = nc.alloc_semaphore("cpa")
    st = nc.alloc_semaphore("st")
    all_sems = lde + [mm_sem, cpv, cpa, st]

    eng = {"s": nc.sync, "a": nc.scalar, "g": nc.gpsimd}

    # entry block: all chunk loads, need-order ring heads
    for i, (a, b) in enumerate(ch):
        ring = eng[LOAD_RINGS[i % len(LOAD_RINGS)]]
        s0 = 0 if i == 0 else CTX + a
        s1 = CTX + b
        ring.dma_start(embx_sb[:, s0:s1], emb_t[i][:, :]).then_inc(lde[i], 16)

    # bank b copy engine: DVE for even b, ACT for odd b
    cp_eng = ["v" if b % 2 == 0 else "a" for b in range(nbk)]
    store_rings = [STORE_RINGS[b % len(STORE_RINGS)] for b in range(nbk)]

    def _bank_copy(b):
        parts, cols = _bank_shape(banks, mm_cols, b)
        src = ps[b][:parts, :cols]
        dst = out_sbs[b][:, :]
        if cp_eng[b] == "v":
            nc.vector.tensor_copy(dst, src).then_inc(cpv, 1)
        else:
            nc.scalar.copy(dst, src).then_inc(cpa, 1)

    def _bank_store(issuer, b):
        # stores wait for the last load so their data transfer never
        # contends with the emb stream (the epilogue drain covers their
        # completion), and for bank b's copy
        issuer.wait_ge(lde[nch - 1], 16)
        n_v = sum(1 for x in cp_eng[: b + 1] if x == "v")
        n_a = b + 1 - n_v
        if cp_eng[b] == "v":
            issuer.wait_ge(cpv, n_v)
        else:
            issuer.wait_ge(cpa, n_a)
        eng[store_rings[b]].dma_start(out_t[b][:, :], out_sbs[b][:, :]).then_inc(
            st, 16
        )

    # flat program (no nc.Block): per-engine streams emitted directly into
    # the entry block -- saves the per-engine branch + the Block-exit
    # drain-barrier. Only per-engine emission order matters.
    for _wi in range(N_WARMUP):
        nc.tensor.matmul(
            ps_w[:CTX, :],
            lhsT=wscr[:, :CTX],
            rhs=wscr[:, CTX:],
            start=True,
            stop=True,
        )
    prev_gate = -1
    for s, (c0, c1) in enumerate(mm_cols):
        if gates[s] != prev_gate:
            nc.tensor.wait_ge(lde[gates[s]], 16)
            prev_gate = gates[s]
        b, q = s // NQ, s % NQ
        nc.tensor.matmul(
            ps[b][32 * q : 32 * q + N, : c1 - c0],
            lhsT=embx_sb[:, :CTX],
            rhs=embx_sb[:, CTX + c0 : CTX + c1],
            start=True,
            stop=True,
            tile_position=(0, 32 * q),
        ).then_inc(mm_sem, 1)

    for b in range(nbk):
        if cp_eng[b] == "v":
            nc.vector.wait_ge(mm_sem, min(banks[b][1], n_mm))
            _bank_copy(b)

    for b in range(nbk):
        if cp_eng[b] == "a":
            nc.scalar.wait_ge(mm_sem, min(banks[b][1], n_mm))
            _bank_copy(b)
        if store_rings[b] == "a":
            _bank_store(nc.scalar, b)

    for b in range(nbk):
        if store_rings[b] == "s":
            _bank_store(nc.sync, b)

    # epilogue: sem-only all-engine barrier, then the sem clear
    # (re-execution safety); the clear's dma_reset drains the store
    # queue so the final writes land.
    nc.all_engine_barrier(sem_only=True)
    nc.clear_and_free_semaphores(all_sems)

    nc.compile()
    return nc


def _get_nc():
    if "nc" not in _cache:
        _cache["nc"] = _build()
    return _cache["nc"]


def _make_in_maps(self_attn, emb_table, value_w):
    bf = ml_dtypes.bfloat16
    sa = np.asarray(self_attn, dtype=np.float32)
    w = np.asarray(value_w, dtype=np.float32)[0]
    ctxT = np.einsum("ntd,t->dn", sa, w).astype(bf)          # [D, N]
    embT = np.asarray(emb_table, dtype=np.float32)[1 : L + 1].T.astype(bf)
    ch, _mm, _g, _banks = _plan()
    maps = []
    for k in range(NCORES):
        shard = embT[:, k * LSH : (k + 1) * LSH]
        m = {}
        for i, (a, b) in enumerate(ch):
            if i == 0:
                blk = np.concatenate([ctxT, shard[:, a:b]], axis=1)
            else:
                blk = shard[:, a:b]
            m[f"emb{i}"] = np.ascontiguousarray(blk)
        maps.append(m)
    return maps


def _unshard(results):
    _ch, mm_cols, _g, banks = _plan()
    full = np.empty((N, L), dtype=np.float32)
    for k in range(NCORES):
        base = k * LSH
        for b, (m0, m1) in enumerate(banks):
            blk = np.asarray(results[k][f"out{b}"]).astype(np.float32)
            for q, s in enumerate(range(m0, m1)):
                c0, c1 = mm_cols[s]
                full[:, base + c0 : base + c1] = blk[
                    32 * q : 32 * q + N, : c1 - c0
                ]
    return full


def run(self_attn, emb_table, value_w, trace=False):
    from concourse.bass_utils import run_bass_kernel_spmd

    nc = _get_nc()
    in_maps = _make_in_maps(self_attn, emb_table, value_w)
    res = run_bass_kernel_spmd(nc, in_maps, list(range(NCORES)), trace=trace)
    return _unshard(res.results), res


def kernel(self_attn, mat2, traj, emb_table, value_w):
    full, _ = run(self_attn, emb_table, value_w, trace=False)
    return full
